# revision 4
# baseline (speedup 1.0000x reference)
"""Multi-head causal attention (B=4, S=2048, D=1024, H=16) on 8 trn2 cores.

Sharding: tensor-parallel over heads x data-parallel over batch.
core c -> (batch b = c//2, head-group hg = c%2 of 8 heads). Every core runs
an identical SPMD program on different data; the host sums the two partial
outputs per batch (the "all-reduce after W_o" done at gather time) and folds
the Wo @ bv + bo constant.

v3 design (vs the 480us f32r baseline):
  - All matmul inputs bf16 (tolerance 2e-2; this lands ~5e-3).
  - Score matmuls for a head PAIR run concurrently in the PE array via
    row tiling: head 2t uses array rows 0-63 (tile_position (0,0)), head
    2t+1 rows 64-127 ((64,0)) - auto-derived from the operands' base
    partitions. Halves score matmul wall time; K/Q tiles need no padding.
  - Weights + K^T/V~/Q/ctx all SBUF-resident; weights load once; ctx never
    round-trips through DRAM.
  - Causal trimming: for diagonal key-block j only query columns >= 128*j
    are computed anywhere (scores/mask/exp/PV). The fine triangular mask is
    applied by accumulating identity.T @ maskbias (0/-30) into the score
    PSUM before exp - no vector-engine mask multiply.
  - Softmax denominators come free as PV row 64 via a ones column in each
    head's V block (stationary M=65). The PV accumulator bank is released
    by a single DVE copy (ctx+denominator -> SBUF); the normalize chain
    (reciprocal, gpsimd broadcast, multiply) then runs entirely off-PSUM
    on otherwise-idle engines. Odd heads' normalized ctx is written with a
    cross-quadrant DVE write (HW-verified) to pack ctx^T for the output
    projection.
  - Q/K projection PSUM->SBUF moves ride on the vector engine
    (tensor_scalar_add with the bias), keeping the scalar engine exp-only.
"""

import sys

import numpy as np

_BASS_PATH = "/opt/trn_rl_repo"
if _BASS_PATH not in sys.path:
    sys.path.insert(0, _BASS_PATH)

B, S, D, H, DK = 4, 2048, 1024, 16, 64
NCORES = 8
FH = 512  # features per core (8 heads)
HL = 8  # local heads
NSC = 4  # seq superblocks of 512
SQ = 512
NDM = 8  # d_model chunks of 128
NEGB = -30.0  # causal mask bias (exp(-30+s) ~ 0 for |s|<=8)

_cache = {}


def _build():
    import concourse.bacc as bacc
    import concourse.mybir as mybir
    from concourse.tile import TileContext

    f32, bf16 = mybir.dt.float32, mybir.dt.bfloat16
    AF = mybir.ActivationFunctionType

    nc = bacc.Bacc("TRN2", target_bir_lowering=False, debug=False, num_devices=1)

    xq_d = nc.dram_tensor("xq", [D, S], bf16, kind="ExternalInput").ap()
    xk_d = nc.dram_tensor("xk", [D, S], bf16, kind="ExternalInput").ap()
    xv_d = nc.dram_tensor("xv", [D, S], bf16, kind="ExternalInput").ap()
    wq_d = nc.dram_tensor("wq", [D, FH], bf16, kind="ExternalInput").ap()
    wk_d = nc.dram_tensor("wk", [D, FH], bf16, kind="ExternalInput").ap()
    wv_d = nc.dram_tensor("wv", [D, FH], bf16, kind="ExternalInput").ap()
    wo_d = nc.dram_tensor("wo", [FH, D], bf16, kind="ExternalInput").ap()
    id_d = nc.dram_tensor("ident", [128, 128], bf16, kind="ExternalInput").ap()
    mb_d = nc.dram_tensor("maskb", [128, 128], bf16, kind="ExternalInput").ap()
    bq_d = nc.dram_tensor("bq", [FH], f32, kind="ExternalInput").ap()
    bk_d = nc.dram_tensor("bk", [FH], f32, kind="ExternalInput").ap()
    out_d = nc.dram_tensor("out", [S, D], f32, kind="ExternalOutput").ap()

    with TileContext(nc) as tc:
        with (
            tc.tile_pool(name="res", bufs=1) as res,
            tc.tile_pool(name="st", bufs=1) as st,
            tc.tile_pool(name="psum", bufs=1, space="PSUM") as psp,
        ):
            # ---- resident tiles ----
            kt = [res.tile([128, S], bf16, name=f"kt{i}", tag=f"kt{i}") for i in range(4)]
            # V~ per key block: 8 heads x (64 V cols + ones col), stride 65
            vaug = [
                res.tile([128, 520], bf16, name=f"va{k}", tag=f"va{k}")
                for k in range(16)
            ]
            wk_sb = [res.tile([128, FH], bf16, name=f"wk{i}", tag=f"wk{i}") for i in range(NDM)]
            wv_sb = [res.tile([128, FH], bf16, name=f"wv{i}", tag=f"wv{i}") for i in range(NDM)]
            wq_sb = [res.tile([128, FH], bf16, name=f"wq{i}", tag=f"wq{i}") for i in range(NDM)]
            w_res = {"k": wk_sb, "v": wv_sb, "q": wq_sb}
            wo_sb = []
            for fc in range(4):
                wt = res.tile([128, D], bf16, name=f"wo{fc}", tag=f"wo{fc}")
                nc.sync.dma_start(wt[:], wo_d[fc * 128 : (fc + 1) * 128, :])
                wo_sb.append(wt)
            for dm in range(NDM):
                nc.sync.dma_start(wk_sb[dm][:], wk_d[dm * 128 : (dm + 1) * 128, :])
                nc.sync.dma_start(wv_sb[dm][:], wv_d[dm * 128 : (dm + 1) * 128, :])
                nc.sync.dma_start(wq_sb[dm][:], wq_d[dm * 128 : (dm + 1) * 128, :])
            ident_t = res.tile([128, 128], bf16, name="ident", tag="ident")
            maskb_t = res.tile([128, 128], bf16, name="maskb", tag="maskb")
            nc.sync.dma_start(ident_t[:], id_d[:])
            nc.sync.dma_start(maskb_t[:], mb_d[:])
            bq_t = [res.tile([128, 1], f32, name=f"bq{i}", tag=f"bq{i}") for i in range(4)]
            bk_t = [res.tile([128, 1], f32, name=f"bk{i}", tag=f"bk{i}") for i in range(4)]
            for i in range(4):
                nc.sync.dma_start(
                    bq_t[i][:],
                    bq_d[i * 128 : (i + 1) * 128].rearrange("(p o) -> p o", o=1),
                )
                nc.sync.dma_start(
                    bk_t[i][:],
                    bk_d[i * 128 : (i + 1) * 128].rearrange("(p o) -> p o", o=1),
                )
            ones_t = res.tile([128, HL], bf16, name="ones", tag="ones")
            nc.vector.memset(ones_t[:], 1.0)
            # Q^T tiles (features x queries), double-buffered over superblocks
            qsf = [
                [
                    res.tile([128, SQ], bf16, name=f"qs{s}_{g}", tag=f"qs{s}_{g}")
                    for g in range(4)
                ]
                for s in range(2)
            ]
            # SBUF-resident ctx^T (features x queries), double-buffered
            cfs = [
                [
                    res.tile([128, SQ], bf16, name=f"cf{s}_{fc}", tag=f"cf{s}_{fc}")
                    for fc in range(4)
                ]
                for s in range(2)
            ]

            def make_proj_thunks(sc):
                thunks = []
                for pname, x_d in (("k", xk_d), ("v", xv_d), ("q", xq_d)):
                    box = {}

                    def load(pname=pname, x_d=x_d, box=box):
                        xr = []
                        for dm in range(NDM):
                            xt = st.tile(
                                [128, SQ], bf16, name=f"x{dm}", tag=f"x{dm}", bufs=1
                            )
                            nc.sync.dma_start(
                                xt[:],
                                x_d[dm * 128 : (dm + 1) * 128, sc * SQ : (sc + 1) * SQ],
                            )
                            xr.append(xt)
                        box["x"] = xr

                    for gi in range(4):

                        def group(pname=pname, gi=gi, box=box, sc=sc, load=load):
                            if gi == 0:
                                load()
                            w_sb = w_res[pname]
                            xr = box["x"]
                            if pname in ("q", "k"):
                                pp = psp.tile(
                                    [128, SQ], f32, name="pp", tag="pp", bufs=2
                                )
                                for dm in range(NDM):
                                    nc.tensor.matmul(
                                        pp[:],
                                        w_sb[dm][:, gi * 128 : (gi + 1) * 128],
                                        xr[dm][:],
                                        start=(dm == 0),
                                        stop=(dm == NDM - 1),
                                    )
                                if pname == "k":
                                    nc.vector.tensor_scalar_add(
                                        kt[gi][:, sc * SQ : (sc + 1) * SQ],
                                        pp[:],
                                        bk_t[gi][:],
                                    )
                                else:
                                    nc.vector.tensor_scalar_add(
                                        qsf[sc % 2][gi][:], pp[:], bq_t[gi][:]
                                    )
                            else:  # v
                                kb = sc * 4 + gi
                                pp = psp.tile(
                                    [128, FH], f32, name="pp", tag="pp", bufs=2
                                )
                                for dm in range(NDM):
                                    nc.tensor.matmul(
                                        pp[:],
                                        xr[dm][:, gi * 128 : (gi + 1) * 128],
                                        w_sb[dm][:],
                                        start=(dm == 0),
                                        stop=(dm == NDM - 1),
                                    )
                                va3 = vaug[kb][:, 0 : HL * 65].rearrange(
                                    "p (h e) -> p h e", e=65
                                )
                                pp3 = pp[:].rearrange("p (h e) -> p h e", e=64)
                                nc.vector.tensor_copy(va3[:, :, 0:64], pp3[:])
                                nc.vector.tensor_copy(
                                    va3[:, :, 64:65],
                                    ones_t[:].rearrange("p (h o) -> p h o", o=1),
                                )

                        thunks.append(group)
                return thunks

            def make_attn_batches(ti, sb):
                """Batches of 4 key-blocks for one head PAIR (2ti, 2ti+1)."""
                nkb = 4 * (sb + 1)
                # off-diagonal blocks first (full N), then diagonal ascending
                kbs = list(range(4 * sb)) + list(range(4 * sb, 4 * sb + 4))
                state = {}

                def finish_head(hh, cp):
                    h = 2 * ti + hh
                    # single DVE copy releases the PSUM bank; everything
                    # after runs off SBUF
                    cu = st.tile([65, SQ], f32, name="cu", tag="cu", bufs=4)
                    nc.vector.tensor_copy(cu[:], cp[0:65, :])
                    d1 = st.tile([1, SQ], f32, name="d1", tag="d1", bufs=4)
                    nc.vector.tensor_copy(d1[:], cu[64:65, :])
                    rc1 = st.tile([1, SQ], f32, name="rc1", tag="rc1", bufs=4)
                    nc.vector.reciprocal_approx_fast(rc1[:], d1[:])
                    rb = st.tile([64, SQ], f32, name="rb", tag="rb", bufs=4)
                    nc.gpsimd.partition_broadcast(rb[:], rc1[:])
                    po = (h % 2) * 64
                    nc.vector.tensor_mul(
                        cfs[sb % 2][ti][po : po + 64, :], cu[0:64, :], rb[:]
                    )

                def batch(b0):
                    if b0 == 0:
                        state["cpA"] = psp.tile(
                            [128, SQ], f32, name="cpA", tag="cpA", bufs=1
                        )
                        state["cpB"] = psp.tile(
                            [128, SQ], f32, name="cpB", tag="cpB", bufs=1
                        )
                        state["emitted"] = 0
                    cps = (state["cpA"], state["cpB"])
                    group = []
                    for i in range(b0, b0 + 4):
                        kb = kbs[i]
                        j = kb - 4 * sb  # >=0 on diagonal blocks
                        c0 = 128 * j if j >= 0 else 0
                        sps = []
                        for hh in range(2):
                            po = hh * 64
                            sp = psp.tile(
                                [128, SQ], f32, name="sp", tag="sp", bufs=4
                            )
                            nc.tensor.matmul(
                                sp[:, c0:SQ],
                                kt[ti][po : po + 64, kb * 128 : (kb + 1) * 128],
                                qsf[sb % 2][ti][po : po + 64, c0:SQ],
                                start=True,
                                stop=(j < 0),
                            )
                            sps.append(sp)
                        if j >= 0:
                            for hh in range(2):
                                nc.tensor.matmul(
                                    sps[hh][:, c0 : c0 + 128],
                                    ident_t[:],
                                    maskb_t[:],
                                    start=False,
                                    stop=True,
                                )
                        ess = []
                        for hh in range(2):
                            es = st.tile(
                                [128, SQ], bf16, name="es", tag="es", bufs=8
                            )
                            nc.scalar.activation(
                                es[:, c0:SQ], sps[hh][:, c0:SQ], AF.Exp
                            )
                            ess.append(es)
                        group.append((kb, c0, ess))
                    for kb, c0, ess in group:
                        for hh in range(2):
                            h = 2 * ti + hh
                            nc.tensor.matmul(
                                cps[hh][0:65, c0:SQ],
                                vaug[kb][:, h * 65 : h * 65 + 65],
                                ess[hh][:, c0:SQ],
                                start=(state["emitted"] == 0),
                                stop=(state["emitted"] == nkb - 1),
                            )
                        state["emitted"] += 1
                    if b0 + 4 >= nkb:
                        finish_head(0, state["cpA"])
                        finish_head(1, state["cpB"])

                return [(lambda b0=b0: batch(b0)) for b0 in range(0, nkb, 4)]

            def make_o_thunks(sb):
                thunks = []
                for qb in range(4):
                    for n2 in range(2):

                        def group(qb=qb, n2=n2, sb=sb):
                            cfc = cfs[sb % 2]
                            pp = psp.tile([128, SQ], f32, name="pp", tag="pp", bufs=2)
                            for fc in range(4):
                                nc.tensor.matmul(
                                    pp[:],
                                    cfc[fc][:, qb * 128 : (qb + 1) * 128],
                                    wo_sb[fc][:, n2 * SQ : (n2 + 1) * SQ],
                                    start=(fc == 0),
                                    stop=(fc == 3),
                                )
                            ob = st.tile([128, SQ], f32, name="ob", tag="ob", bufs=2)
                            nc.vector.tensor_copy(ob[:], pp[:])
                            nc.sync.dma_start(
                                out_d[
                                    sb * SQ + qb * 128 : sb * SQ + (qb + 1) * 128,
                                    n2 * SQ : (n2 + 1) * SQ,
                                ],
                                ob[:],
                            )

                        thunks.append(group)
                return thunks

            # ---- emission schedule ----
            for t in make_proj_thunks(0):
                t()
            for sb in range(NSC):
                batches = []
                for ti in range(4):
                    batches += make_attn_batches(ti, sb)
                warm = []
                if sb < NSC - 1:
                    warm += make_proj_thunks(sb + 1)
                if sb >= 1:
                    warm += make_o_thunks(sb - 1)
                nb, nw = len(batches), len(warm)
                wi = 0
                for bi, bt in enumerate(batches):
                    bt()
                    while wi < nw and (wi + 1) * nb <= (bi + 1) * nw:
                        warm[wi]()
                        wi += 1
                while wi < nw:
                    warm[wi]()
                    wi += 1
            for t in make_o_thunks(NSC - 1):
                t()

    nc.compile()
    return nc


def kernel(
    q,
    k,
    v,
    mask=None,
    Wq=None,
    bq=None,
    Wk=None,
    bk=None,
    Wv=None,
    bv=None,
    Wo=None,
    bo=None,
    **_unused,
):
    import ml_dtypes

    from concourse.bass_utils import run_bass_kernel_spmd

    if "nc" not in _cache:
        _cache["nc"] = _build()
    nc = _cache["nc"]

    bf16 = ml_dtypes.bfloat16
    q = np.asarray(q, np.float32)
    k = np.asarray(k, np.float32)
    v = np.asarray(v, np.float32)
    Wq = np.asarray(Wq, np.float32)
    Wk = np.asarray(Wk, np.float32)
    Wv = np.asarray(Wv, np.float32)
    Wo = np.asarray(Wo, np.float32)
    bq = np.zeros(D, np.float32) if bq is None else np.asarray(bq, np.float32)
    bk = np.zeros(D, np.float32) if bk is None else np.asarray(bk, np.float32)
    bv = np.zeros(D, np.float32) if bv is None else np.asarray(bv, np.float32)
    bo = np.zeros(D, np.float32) if bo is None else np.asarray(bo, np.float32)

    ident = np.eye(128, dtype=np.float32).astype(bf16)
    kk = np.arange(128)[:, None]
    qq = np.arange(128)[None, :]
    maskb = np.where(kk <= qq, 0.0, NEGB).astype(bf16)

    xT = {}
    for b in range(B):
        xT[("q", b)] = np.ascontiguousarray(q[b].T).astype(bf16)
        xT[("k", b)] = np.ascontiguousarray(k[b].T).astype(bf16)
        xT[("v", b)] = np.ascontiguousarray(v[b].T).astype(bf16)
    wqs, wks, wvs, wos, bqs, bks = {}, {}, {}, {}, {}, {}
    for hg in range(2):
        sl = slice(hg * FH, (hg + 1) * FH)
        wqs[hg] = (np.ascontiguousarray(Wq[sl, :].T) * np.float32(0.125)).astype(bf16)
        wks[hg] = np.ascontiguousarray(Wk[sl, :].T).astype(bf16)
        wvs[hg] = np.ascontiguousarray(Wv[sl, :].T).astype(bf16)
        wos[hg] = np.ascontiguousarray(Wo[:, sl].T).astype(bf16)
        bqs[hg] = np.ascontiguousarray(bq[sl]) * np.float32(0.125)
        bks[hg] = np.ascontiguousarray(bk[sl])

    in_maps = []
    for c in range(NCORES):
        b, hg = c // 2, c % 2
        in_maps.append(
            {
                "xq": xT[("q", b)],
                "xk": xT[("k", b)],
                "xv": xT[("v", b)],
                "wq": wqs[hg],
                "wk": wks[hg],
                "wv": wvs[hg],
                "wo": wos[hg],
                "ident": ident,
                "maskb": maskb,
                "bq": bqs[hg],
                "bk": bks[hg],
            }
        )

    res = run_bass_kernel_spmd(nc, in_maps, list(range(NCORES)))
    out = np.empty((B, S, D), np.float32)
    for b in range(B):
        out[b] = res.results[2 * b]["out"] + res.results[2 * b + 1]["out"]
    const = Wo @ bv + bo  # bv/bo contribution (folds exactly through softmax)
    if np.any(const):
        out += const[None, None, :]
    return out


# revision 8
# speedup vs baseline: 1.1800x; 1.1800x over previous
"""Multi-head causal attention (B=4, S=2048, D=1024, H=16) on 8 trn2 cores.

Sharding: tensor-parallel over heads x data-parallel over batch.
core c -> (batch b = c//2, head-group hg = c%2 of 8 heads). Every core runs
an identical SPMD program on different data; the host sums the two partial
outputs per batch (the "all-reduce after W_o" done at gather time) and folds
the Wo @ bv + bo constant.

v3 design (vs the 480us f32r baseline):
  - All matmul inputs bf16 (tolerance 2e-2; this lands ~5e-3).
  - Score matmuls for a head PAIR run concurrently in the PE array via
    row tiling: head 2t uses array rows 0-63 (tile_position (0,0)), head
    2t+1 rows 64-127 ((64,0)) - auto-derived from the operands' base
    partitions. Halves score matmul wall time; K/Q tiles need no padding.
  - Weights + K^T/V~/Q/ctx all SBUF-resident; weights load once; ctx never
    round-trips through DRAM.
  - Causal trimming: for diagonal key-block j only query columns >= 128*j
    are computed anywhere (scores/mask/exp/PV). The fine triangular mask is
    applied by accumulating identity.T @ maskbias (0/-30) into the score
    PSUM before exp - no vector-engine mask multiply.
  - Softmax denominators come free as PV row 64 via a ones column in each
    head's V block (stationary M=65). The PV accumulator bank is released
    by a single DVE copy (ctx+denominator -> SBUF); the normalize chain
    (reciprocal, gpsimd broadcast, multiply) then runs entirely off-PSUM
    on otherwise-idle engines. Odd heads' normalized ctx is written with a
    cross-quadrant DVE write (HW-verified) to pack ctx^T for the output
    projection.
  - Q/K projection PSUM->SBUF moves ride on the vector engine
    (tensor_scalar_add with the bias), keeping the scalar engine exp-only.
"""

import sys

import numpy as np

_BASS_PATH = "/opt/trn_rl_repo"
if _BASS_PATH not in sys.path:
    sys.path.insert(0, _BASS_PATH)

B, S, D, H, DK = 4, 2048, 1024, 16, 64
NCORES = 8
FH = 512  # features per core (8 heads)
HL = 8  # local heads
NSC = 4  # seq superblocks of 512
SQ = 512
NDM = 8  # d_model chunks of 128
NEGB = -30.0  # causal mask bias (exp(-30+s) ~ 0 for |s|<=8)

_cache = {}


def _build():
    import concourse.bacc as bacc
    import concourse.mybir as mybir
    from concourse.tile import TileContext

    f32, bf16 = mybir.dt.float32, mybir.dt.bfloat16
    AF = mybir.ActivationFunctionType

    nc = bacc.Bacc("TRN2", target_bir_lowering=False, debug=False, num_devices=1)

    xq_d = nc.dram_tensor("xq", [D, S], bf16, kind="ExternalInput").ap()
    xk_d = nc.dram_tensor("xk", [D, S], bf16, kind="ExternalInput").ap()
    xv_d = nc.dram_tensor("xv", [D, S], bf16, kind="ExternalInput").ap()
    wq_d = nc.dram_tensor("wq", [D, FH], bf16, kind="ExternalInput").ap()
    wk_d = nc.dram_tensor("wk", [D, FH], bf16, kind="ExternalInput").ap()
    wv_d = nc.dram_tensor("wv", [D, FH], bf16, kind="ExternalInput").ap()
    wo_d = nc.dram_tensor("wo", [FH, D], bf16, kind="ExternalInput").ap()
    id_d = nc.dram_tensor("ident", [128, 128], bf16, kind="ExternalInput").ap()
    mb_d = nc.dram_tensor("maskb", [128, 128], bf16, kind="ExternalInput").ap()
    bq_d = nc.dram_tensor("bq", [FH], f32, kind="ExternalInput").ap()
    bk_d = nc.dram_tensor("bk", [FH], f32, kind="ExternalInput").ap()
    out_d = nc.dram_tensor("out", [S, D], f32, kind="ExternalOutput").ap()

    with TileContext(nc) as tc:
        with (
            tc.tile_pool(name="res", bufs=1) as res,
            tc.tile_pool(name="st", bufs=1) as st,
            tc.tile_pool(name="psum", bufs=1, space="PSUM") as psp,
        ):
            # ---- resident tiles ----
            kt = [res.tile([128, S], bf16, name=f"kt{i}", tag=f"kt{i}") for i in range(4)]
            # V~ per key block: 8 heads x (64 V cols + ones col), stride 65
            vaug = [
                res.tile([128, 520], bf16, name=f"va{k}", tag=f"va{k}")
                for k in range(16)
            ]
            wk_sb = [res.tile([128, FH], bf16, name=f"wk{i}", tag=f"wk{i}") for i in range(NDM)]
            wv_sb = [res.tile([128, FH], bf16, name=f"wv{i}", tag=f"wv{i}") for i in range(NDM)]
            wq_sb = [res.tile([128, FH], bf16, name=f"wq{i}", tag=f"wq{i}") for i in range(NDM)]
            w_res = {"k": wk_sb, "v": wv_sb, "q": wq_sb}
            wo_sb = []
            for fc in range(4):
                wt = res.tile([128, D], bf16, name=f"wo{fc}", tag=f"wo{fc}")
                nc.sync.dma_start(wt[:], wo_d[fc * 128 : (fc + 1) * 128, :])
                wo_sb.append(wt)
            for dm in range(NDM):
                nc.sync.dma_start(wk_sb[dm][:], wk_d[dm * 128 : (dm + 1) * 128, :])
                nc.sync.dma_start(wv_sb[dm][:], wv_d[dm * 128 : (dm + 1) * 128, :])
                nc.sync.dma_start(wq_sb[dm][:], wq_d[dm * 128 : (dm + 1) * 128, :])
            ident_t = res.tile([128, 128], bf16, name="ident", tag="ident")
            maskb_t = res.tile([128, 128], bf16, name="maskb", tag="maskb")
            nc.sync.dma_start(ident_t[:], id_d[:])
            nc.sync.dma_start(maskb_t[:], mb_d[:])
            bq_t = [res.tile([128, 1], f32, name=f"bq{i}", tag=f"bq{i}") for i in range(4)]
            bk_t = [res.tile([128, 1], f32, name=f"bk{i}", tag=f"bk{i}") for i in range(4)]
            for i in range(4):
                nc.sync.dma_start(
                    bq_t[i][:],
                    bq_d[i * 128 : (i + 1) * 128].rearrange("(p o) -> p o", o=1),
                )
                nc.sync.dma_start(
                    bk_t[i][:],
                    bk_d[i * 128 : (i + 1) * 128].rearrange("(p o) -> p o", o=1),
                )
            ones_t = res.tile([128, HL], bf16, name="ones", tag="ones")
            nc.vector.memset(ones_t[:], 1.0)
            # per-head zero-padded Q tiles, double-buffered over superblocks:
            # head h data lives in rows (h%2)*64 .. +64, other 64 rows are 0,
            # so score matmuls contract K=128 (keeps the HAM clock-gate warm)
            qz = [
                [
                    res.tile([128, SQ], bf16, name=f"qz{s}_{h}", tag=f"qz{s}_{h}")
                    for h in range(HL)
                ]
                for s in range(2)
            ]
            for s in range(2):
                for h in range(HL):
                    zr = 64 if (h % 2 == 0) else 0
                    nc.vector.memset(qz[s][h][zr : zr + 64, :], 0.0)
            # SBUF-resident ctx^T (features x queries), double-buffered
            cfs = [
                [
                    res.tile([128, SQ], bf16, name=f"cf{s}_{fc}", tag=f"cf{s}_{fc}")
                    for fc in range(4)
                ]
                for s in range(2)
            ]

            def make_proj_thunks(sc):
                thunks = []
                for pname, x_d in (("k", xk_d), ("v", xv_d), ("q", xq_d)):
                    box = {}

                    def load(pname=pname, x_d=x_d, box=box):
                        xr = []
                        for dm in range(NDM):
                            xt = st.tile(
                                [128, SQ], bf16, name=f"x{dm}", tag=f"x{dm}", bufs=1
                            )
                            nc.sync.dma_start(
                                xt[:],
                                x_d[dm * 128 : (dm + 1) * 128, sc * SQ : (sc + 1) * SQ],
                            )
                            xr.append(xt)
                        box["x"] = xr

                    for gi in range(4):

                        def group(pname=pname, gi=gi, box=box, sc=sc, load=load):
                            if gi == 0:
                                load()
                            w_sb = w_res[pname]
                            xr = box["x"]
                            if pname in ("q", "k"):
                                pp = psp.tile(
                                    [128, SQ], f32, name="pp", tag="pp", bufs=2
                                )
                                for dm in range(NDM):
                                    nc.tensor.matmul(
                                        pp[:],
                                        w_sb[dm][:, gi * 128 : (gi + 1) * 128],
                                        xr[dm][:],
                                        start=(dm == 0),
                                        stop=(dm == NDM - 1),
                                    )
                                if pname == "k":
                                    nc.vector.tensor_scalar_add(
                                        kt[gi][:, sc * SQ : (sc + 1) * SQ],
                                        pp[:],
                                        bk_t[gi][:],
                                    )
                                else:
                                    s = sc % 2
                                    nc.vector.tensor_scalar_add(
                                        qz[s][2 * gi][0:64, :],
                                        pp[0:64, :],
                                        bq_t[gi][0:64],
                                    )
                                    nc.vector.tensor_scalar_add(
                                        qz[s][2 * gi + 1][64:128, :],
                                        pp[64:128, :],
                                        bq_t[gi][64:128],
                                    )
                            else:  # v
                                kb = sc * 4 + gi
                                pp = psp.tile(
                                    [128, FH], f32, name="pp", tag="pp", bufs=2
                                )
                                for dm in range(NDM):
                                    nc.tensor.matmul(
                                        pp[:],
                                        xr[dm][:, gi * 128 : (gi + 1) * 128],
                                        w_sb[dm][:],
                                        start=(dm == 0),
                                        stop=(dm == NDM - 1),
                                    )
                                va3 = vaug[kb][:, 0 : HL * 65].rearrange(
                                    "p (h e) -> p h e", e=65
                                )
                                pp3 = pp[:].rearrange("p (h e) -> p h e", e=64)
                                nc.vector.tensor_copy(va3[:, :, 0:64], pp3[:])
                                nc.vector.tensor_copy(
                                    va3[:, :, 64:65],
                                    ones_t[:].rearrange("p (h o) -> p h o", o=1),
                                )

                        thunks.append(group)
                return thunks

            def make_attn_batches(h, sb):
                """Batches of 4 key-blocks for one (head, superblock)."""
                ti = h // 2
                nkb = 4 * (sb + 1)
                # off-diagonal blocks first (full N), then diagonal ascending
                kbs = list(range(4 * sb)) + list(range(4 * sb, 4 * sb + 4))
                state = {}

                def batch(b0):
                    if b0 == 0:
                        state["cp"] = psp.tile(
                            [128, SQ], f32, name="cp", tag="cp", bufs=2
                        )
                        state["emitted"] = 0
                    cp = state["cp"]
                    group = []
                    for i in range(b0, b0 + 4):
                        kb = kbs[i]
                        j = kb - 4 * sb  # >=0 on diagonal blocks
                        c0 = 128 * j if j >= 0 else 0
                        sp = psp.tile([128, SQ], f32, name="sp", tag="sp", bufs=4)
                        nc.tensor.matmul(
                            sp[:, c0:SQ],
                            kt[ti][:, kb * 128 : (kb + 1) * 128],
                            qz[sb % 2][h][:, c0:SQ],
                            start=True,
                            stop=(j < 0),
                        )
                        if j >= 0:
                            nc.tensor.matmul(
                                sp[:, c0 : c0 + 128],
                                ident_t[:],
                                maskb_t[:],
                                start=False,
                                stop=True,
                            )
                        es = st.tile([128, SQ], bf16, name="es", tag="es", bufs=6)
                        nc.scalar.activation(es[:, c0:SQ], sp[:, c0:SQ], AF.Exp)
                        group.append((kb, c0, es))
                    for kb, c0, es in group:
                        nc.tensor.matmul(
                            cp[0:65, c0:SQ],
                            vaug[kb][:, h * 65 : h * 65 + 65],
                            es[:, c0:SQ],
                            start=(state["emitted"] == 0),
                            stop=(state["emitted"] == nkb - 1),
                        )
                        state["emitted"] += 1
                    if b0 + 4 >= nkb:
                        # single DVE copy releases the PSUM bank; the rest of
                        # the normalize chain runs off SBUF on idle engines
                        cu = st.tile([65, SQ], f32, name="cu", tag="cu", bufs=4)
                        nc.vector.tensor_copy(cu[:], cp[0:65, :])
                        d1 = st.tile([1, SQ], f32, name="d1", tag="d1", bufs=4)
                        nc.vector.tensor_copy(d1[:], cu[64:65, :])
                        rc1 = st.tile([1, SQ], f32, name="rc1", tag="rc1", bufs=4)
                        nc.vector.reciprocal_approx_fast(rc1[:], d1[:])
                        rb = st.tile([64, SQ], f32, name="rb", tag="rb", bufs=4)
                        nc.gpsimd.partition_broadcast(rb[:], rc1[:])
                        po = (h % 2) * 64
                        nc.vector.tensor_mul(
                            cfs[sb % 2][ti][po : po + 64, :], cu[0:64, :], rb[:]
                        )

                return [(lambda b0=b0: batch(b0)) for b0 in range(0, nkb, 4)]

            def make_o_thunks(sb):
                thunks = []
                for qb in range(4):
                    for n2 in range(2):

                        def group(qb=qb, n2=n2, sb=sb):
                            cfc = cfs[sb % 2]
                            pp = psp.tile([128, SQ], f32, name="pp", tag="pp", bufs=2)
                            for fc in range(4):
                                nc.tensor.matmul(
                                    pp[:],
                                    cfc[fc][:, qb * 128 : (qb + 1) * 128],
                                    wo_sb[fc][:, n2 * SQ : (n2 + 1) * SQ],
                                    start=(fc == 0),
                                    stop=(fc == 3),
                                )
                            ob = st.tile([128, SQ], f32, name="ob", tag="ob", bufs=2)
                            nc.vector.tensor_copy(ob[:], pp[:])
                            nc.sync.dma_start(
                                out_d[
                                    sb * SQ + qb * 128 : sb * SQ + (qb + 1) * 128,
                                    n2 * SQ : (n2 + 1) * SQ,
                                ],
                                ob[:],
                            )

                        thunks.append(group)
                return thunks

            # ---- emission schedule ----
            for t in make_proj_thunks(0):
                t()
            for sb in range(NSC):
                batches = []
                for h in range(HL):
                    batches += make_attn_batches(h, sb)
                warm = []
                if sb < NSC - 1:
                    warm += make_proj_thunks(sb + 1)
                if sb >= 1:
                    warm += make_o_thunks(sb - 1)
                nb, nw = len(batches), len(warm)
                wi = 0
                for bi, bt in enumerate(batches):
                    bt()
                    while wi < nw and (wi + 1) * nb <= (bi + 1) * nw:
                        warm[wi]()
                        wi += 1
                while wi < nw:
                    warm[wi]()
                    wi += 1
            for t in make_o_thunks(NSC - 1):
                t()

    nc.compile()
    return nc


def kernel(
    q,
    k,
    v,
    mask=None,
    Wq=None,
    bq=None,
    Wk=None,
    bk=None,
    Wv=None,
    bv=None,
    Wo=None,
    bo=None,
    **_unused,
):
    import ml_dtypes

    from concourse.bass_utils import run_bass_kernel_spmd

    if "nc" not in _cache:
        _cache["nc"] = _build()
    nc = _cache["nc"]

    bf16 = ml_dtypes.bfloat16
    q = np.asarray(q, np.float32)
    k = np.asarray(k, np.float32)
    v = np.asarray(v, np.float32)
    Wq = np.asarray(Wq, np.float32)
    Wk = np.asarray(Wk, np.float32)
    Wv = np.asarray(Wv, np.float32)
    Wo = np.asarray(Wo, np.float32)
    bq = np.zeros(D, np.float32) if bq is None else np.asarray(bq, np.float32)
    bk = np.zeros(D, np.float32) if bk is None else np.asarray(bk, np.float32)
    bv = np.zeros(D, np.float32) if bv is None else np.asarray(bv, np.float32)
    bo = np.zeros(D, np.float32) if bo is None else np.asarray(bo, np.float32)

    ident = np.eye(128, dtype=np.float32).astype(bf16)
    kk = np.arange(128)[:, None]
    qq = np.arange(128)[None, :]
    maskb = np.where(kk <= qq, 0.0, NEGB).astype(bf16)

    xT = {}
    for b in range(B):
        xT[("q", b)] = np.ascontiguousarray(q[b].T).astype(bf16)
        xT[("k", b)] = np.ascontiguousarray(k[b].T).astype(bf16)
        xT[("v", b)] = np.ascontiguousarray(v[b].T).astype(bf16)
    wqs, wks, wvs, wos, bqs, bks = {}, {}, {}, {}, {}, {}
    for hg in range(2):
        sl = slice(hg * FH, (hg + 1) * FH)
        wqs[hg] = (np.ascontiguousarray(Wq[sl, :].T) * np.float32(0.125)).astype(bf16)
        wks[hg] = np.ascontiguousarray(Wk[sl, :].T).astype(bf16)
        wvs[hg] = np.ascontiguousarray(Wv[sl, :].T).astype(bf16)
        wos[hg] = np.ascontiguousarray(Wo[:, sl].T).astype(bf16)
        bqs[hg] = np.ascontiguousarray(bq[sl]) * np.float32(0.125)
        bks[hg] = np.ascontiguousarray(bk[sl])

    in_maps = []
    for c in range(NCORES):
        b, hg = c // 2, c % 2
        in_maps.append(
            {
                "xq": xT[("q", b)],
                "xk": xT[("k", b)],
                "xv": xT[("v", b)],
                "wq": wqs[hg],
                "wk": wks[hg],
                "wv": wvs[hg],
                "wo": wos[hg],
                "ident": ident,
                "maskb": maskb,
                "bq": bqs[hg],
                "bk": bks[hg],
            }
        )

    res = run_bass_kernel_spmd(nc, in_maps, list(range(NCORES)))
    out = np.empty((B, S, D), np.float32)
    for b in range(B):
        out[b] = res.results[2 * b]["out"] + res.results[2 * b + 1]["out"]
    const = Wo @ bv + bo  # bv/bo contribution (folds exactly through softmax)
    if np.any(const):
        out += const[None, None, :]
    return out


# revision 12
# speedup vs baseline: 1.2378x; 1.0490x over previous
"""Multi-head causal attention (B=4, S=2048, D=1024, H=16) on 8 trn2 cores.

Sharding: tensor-parallel over heads x data-parallel over batch.
core c -> (batch b = c//2, head-group hg = c%2 of 8 heads). Every core runs
an identical SPMD program on different data; the host sums the two partial
outputs per batch (the "all-reduce after W_o" done at gather time) and folds
the Wo @ bv + bo constant.

v3 design (vs the 480us f32r baseline):
  - All matmul inputs bf16 (tolerance 2e-2; this lands ~5e-3).
  - Score matmuls for a head PAIR run concurrently in the PE array via
    row tiling: head 2t uses array rows 0-63 (tile_position (0,0)), head
    2t+1 rows 64-127 ((64,0)) - auto-derived from the operands' base
    partitions. Halves score matmul wall time; K/Q tiles need no padding.
  - Weights + K^T/V~/Q/ctx all SBUF-resident; weights load once; ctx never
    round-trips through DRAM.
  - Causal trimming: for diagonal key-block j only query columns >= 128*j
    are computed anywhere (scores/mask/exp/PV). The fine triangular mask is
    applied by accumulating identity.T @ maskbias (0/-30) into the score
    PSUM before exp - no vector-engine mask multiply.
  - Softmax denominators come free as PV row 64 via a ones column in each
    head's V block (stationary M=65). The PV accumulator bank is released
    by a single DVE copy (ctx+denominator -> SBUF); the normalize chain
    (reciprocal, gpsimd broadcast, multiply) then runs entirely off-PSUM
    on otherwise-idle engines. Odd heads' normalized ctx is written with a
    cross-quadrant DVE write (HW-verified) to pack ctx^T for the output
    projection.
  - Q/K projection PSUM->SBUF moves ride on the vector engine
    (tensor_scalar_add with the bias), keeping the scalar engine exp-only.
"""

import sys

import numpy as np

_BASS_PATH = "/opt/trn_rl_repo"
if _BASS_PATH not in sys.path:
    sys.path.insert(0, _BASS_PATH)

B, S, D, H, DK = 4, 2048, 1024, 16, 64
NCORES = 8
FH = 512  # features per core (8 heads)
HL = 8  # local heads
NSC = 4  # seq superblocks of 512
SQ = 512
NDM = 8  # d_model chunks of 128
NEGB = -30.0  # causal mask bias (exp(-30+s) ~ 0 for |s|<=8)

_cache = {}


def _build():
    import concourse.bacc as bacc
    import concourse.mybir as mybir
    from concourse.tile import TileContext

    f32, bf16 = mybir.dt.float32, mybir.dt.bfloat16
    AF = mybir.ActivationFunctionType

    nc = bacc.Bacc("TRN2", target_bir_lowering=False, debug=False, num_devices=1)

    xq_d = nc.dram_tensor("xq", [D, S], bf16, kind="ExternalInput").ap()
    xk_d = nc.dram_tensor("xk", [D, S], bf16, kind="ExternalInput").ap()
    xv_d = nc.dram_tensor("xv", [D, S], bf16, kind="ExternalInput").ap()
    wq_d = nc.dram_tensor("wq", [D, FH], bf16, kind="ExternalInput").ap()
    wk_d = nc.dram_tensor("wk", [D, FH], bf16, kind="ExternalInput").ap()
    wv_d = nc.dram_tensor("wv", [D, FH], bf16, kind="ExternalInput").ap()
    wo_d = nc.dram_tensor("wo", [FH, D], bf16, kind="ExternalInput").ap()
    id_d = nc.dram_tensor("ident", [128, 128], bf16, kind="ExternalInput").ap()
    mb_d = nc.dram_tensor("maskb", [128, 128], bf16, kind="ExternalInput").ap()
    bq_d = nc.dram_tensor("bq", [FH], f32, kind="ExternalInput").ap()
    bk_d = nc.dram_tensor("bk", [FH], f32, kind="ExternalInput").ap()
    out_d = nc.dram_tensor("out", [S, D], f32, kind="ExternalOutput").ap()

    with TileContext(nc) as tc:
        with (
            tc.tile_pool(name="res", bufs=1) as res,
            tc.tile_pool(name="st", bufs=1) as st,
            tc.tile_pool(name="psum", bufs=1, space="PSUM") as psp,
        ):
            # ---- resident tiles (DMAs emitted later in priority order) ----
            kt = [res.tile([128, S], bf16, name=f"kt{i}", tag=f"kt{i}") for i in range(4)]
            # V~ per key block: 8 heads x (64 V cols + ones col), stride 65
            vaug = [
                res.tile([128, 520], bf16, name=f"va{k}", tag=f"va{k}")
                for k in range(16)
            ]
            wk_sb = [res.tile([128, FH], bf16, name=f"wk{i}", tag=f"wk{i}") for i in range(NDM)]
            wv_sb = [res.tile([128, FH], bf16, name=f"wv{i}", tag=f"wv{i}") for i in range(NDM)]
            wq_sb = [res.tile([128, FH], bf16, name=f"wq{i}", tag=f"wq{i}") for i in range(NDM)]
            w_res = {"k": wk_sb, "v": wv_sb, "q": wq_sb}
            w_dram = {"k": wk_d, "v": wv_d, "q": wq_d}
            wo_sb = [
                res.tile([128, D], bf16, name=f"wo{fc}", tag=f"wo{fc}")
                for fc in range(4)
            ]
            ident_t = res.tile([128, 128], bf16, name="ident", tag="ident")
            maskb_t = res.tile([128, 128], bf16, name="maskb", tag="maskb")
            bq_t = [res.tile([128, 1], f32, name=f"bq{i}", tag=f"bq{i}") for i in range(4)]
            bk_t = [res.tile([128, 1], f32, name=f"bk{i}", tag=f"bk{i}") for i in range(4)]
            ones_t = res.tile([128, HL], bf16, name="ones", tag="ones")
            nc.vector.memset(ones_t[:], 1.0)

            def load_w(pname):
                for dm in range(NDM):
                    nc.sync.dma_start(
                        w_res[pname][dm][:],
                        w_dram[pname][dm * 128 : (dm + 1) * 128, :],
                    )

            def load_bias(bt, b_d):
                for i in range(4):
                    nc.sync.dma_start(
                        bt[i][:],
                        b_d[i * 128 : (i + 1) * 128].rearrange("(p o) -> p o", o=1),
                    )

            def load_misc():
                nc.sync.dma_start(ident_t[:], id_d[:])
                nc.sync.dma_start(maskb_t[:], mb_d[:])
                for fc in range(4):
                    nc.sync.dma_start(wo_sb[fc][:], wo_d[fc * 128 : (fc + 1) * 128, :])
            # per-head zero-padded Q tiles, double-buffered over superblocks:
            # head h data lives in rows (h%2)*64 .. +64, other 64 rows are 0,
            # so score matmuls contract K=128 (keeps the HAM clock-gate warm)
            qz = [
                [
                    res.tile([128, SQ], bf16, name=f"qz{s}_{h}", tag=f"qz{s}_{h}")
                    for h in range(HL)
                ]
                for s in range(2)
            ]
            for s in range(2):
                for h in range(HL):
                    zr = 64 if (h % 2 == 0) else 0
                    nc.vector.memset(qz[s][h][zr : zr + 64, :], 0.0)
            # SBUF-resident ctx^T (features x queries), double-buffered
            cfs = [
                [
                    res.tile([128, SQ], bf16, name=f"cf{s}_{fc}", tag=f"cf{s}_{fc}")
                    for fc in range(4)
                ]
                for s in range(2)
            ]

            def make_proj_thunks(sc, only=None):
                thunks = []
                for pname, x_d in (("k", xk_d), ("v", xv_d), ("q", xq_d)):
                    if only is not None and pname != only:
                        continue
                    box = {}

                    def load(pname=pname, x_d=x_d, box=box):
                        xr = []
                        for dm in range(NDM):
                            xt = st.tile(
                                [128, SQ], bf16, name=f"x{dm}", tag=f"x{dm}", bufs=1
                            )
                            nc.sync.dma_start(
                                xt[:],
                                x_d[dm * 128 : (dm + 1) * 128, sc * SQ : (sc + 1) * SQ],
                            )
                            xr.append(xt)
                        box["x"] = xr

                    for gi in range(4):

                        def group(pname=pname, gi=gi, box=box, sc=sc, load=load):
                            if gi == 0:
                                load()
                            w_sb = w_res[pname]
                            xr = box["x"]
                            if pname in ("q", "k"):
                                pp = psp.tile(
                                    [128, SQ], f32, name="pp", tag="pp", bufs=2
                                )
                                for dm in range(NDM):
                                    nc.tensor.matmul(
                                        pp[:],
                                        w_sb[dm][:, gi * 128 : (gi + 1) * 128],
                                        xr[dm][:],
                                        start=(dm == 0),
                                        stop=(dm == NDM - 1),
                                    )
                                if pname == "k":
                                    nc.vector.tensor_scalar_add(
                                        kt[gi][:, sc * SQ : (sc + 1) * SQ],
                                        pp[:],
                                        bk_t[gi][:],
                                    )
                                else:
                                    s = sc % 2
                                    nc.vector.tensor_scalar_add(
                                        qz[s][2 * gi][0:64, :],
                                        pp[0:64, :],
                                        bq_t[gi][0:64],
                                    )
                                    nc.vector.tensor_scalar_add(
                                        qz[s][2 * gi + 1][64:128, :],
                                        pp[64:128, :],
                                        bq_t[gi][64:128],
                                    )
                            else:  # v
                                kb = sc * 4 + gi
                                pp = psp.tile(
                                    [128, FH], f32, name="pp", tag="pp", bufs=2
                                )
                                for dm in range(NDM):
                                    nc.tensor.matmul(
                                        pp[:],
                                        xr[dm][:, gi * 128 : (gi + 1) * 128],
                                        w_sb[dm][:],
                                        start=(dm == 0),
                                        stop=(dm == NDM - 1),
                                    )
                                va3 = vaug[kb][:, 0 : HL * 65].rearrange(
                                    "p (h e) -> p h e", e=65
                                )
                                pp3 = pp[:].rearrange("p (h e) -> p h e", e=64)
                                nc.vector.tensor_copy(va3[:, :, 0:64], pp3[:])
                                nc.vector.tensor_copy(
                                    va3[:, :, 64:65],
                                    ones_t[:].rearrange("p (h o) -> p h o", o=1),
                                )

                        thunks.append(group)
                return thunks

            def make_attn_stream(h, sb):
                """(score-group, pv-group) thunk pairs for one (head, sb).

                Score groups (QK matmuls + mask + exp) and PV groups are
                emitted with a one-group lag by the scheduler so the PE never
                waits on the scalar engine's exp latency.
                """
                ti = h // 2
                nkb = 4 * (sb + 1)
                # off-diagonal blocks first (full N), then diagonal ascending
                kbs = list(range(4 * sb)) + list(range(4 * sb, 4 * sb + 4))
                state = {}

                def sg(b0):
                    group = []
                    for i in range(b0, b0 + 4):
                        kb = kbs[i]
                        j = kb - 4 * sb  # >=0 on diagonal blocks
                        c0 = 128 * j if j >= 0 else 0
                        sp = psp.tile([128, SQ], f32, name="sp", tag="sp", bufs=4)
                        nc.tensor.matmul(
                            sp[:, c0:SQ],
                            kt[ti][:, kb * 128 : (kb + 1) * 128],
                            qz[sb % 2][h][:, c0:SQ],
                            start=True,
                            stop=(j < 0),
                        )
                        if j >= 0:
                            nc.tensor.matmul(
                                sp[:, c0 : c0 + 128],
                                ident_t[:],
                                maskb_t[:],
                                start=False,
                                stop=True,
                            )
                        es = st.tile([128, SQ], bf16, name="es", tag="es", bufs=8)
                        nc.scalar.activation(es[:, c0:SQ], sp[:, c0:SQ], AF.Exp)
                        group.append((kb, c0, es))
                    state[b0] = group

                def pg(b0):
                    if b0 == 0:
                        state["cp"] = psp.tile(
                            [128, SQ], f32, name="cp", tag="cp", bufs=2
                        )
                        state["emitted"] = 0
                    cp = state["cp"]
                    for kb, c0, es in state.pop(b0):
                        nc.tensor.matmul(
                            cp[0:65, c0:SQ],
                            vaug[kb][:, h * 65 : h * 65 + 65],
                            es[:, c0:SQ],
                            start=(state["emitted"] == 0),
                            stop=(state["emitted"] == nkb - 1),
                        )
                        state["emitted"] += 1
                    if b0 + 4 >= nkb:
                        # single DVE copy releases the PSUM bank; the rest of
                        # the normalize chain runs off SBUF on idle engines
                        cu = st.tile([65, SQ], f32, name="cu", tag="cu", bufs=4)
                        nc.vector.tensor_copy(cu[:], cp[0:65, :])
                        d1 = st.tile([1, SQ], f32, name="d1", tag="d1", bufs=4)
                        nc.vector.tensor_copy(d1[:], cu[64:65, :])
                        rc1 = st.tile([1, SQ], f32, name="rc1", tag="rc1", bufs=4)
                        nc.vector.reciprocal_approx_fast(rc1[:], d1[:])
                        rb = st.tile([64, SQ], f32, name="rb", tag="rb", bufs=4)
                        nc.gpsimd.partition_broadcast(rb[:], rc1[:])
                        po = (h % 2) * 64
                        nc.vector.tensor_mul(
                            cfs[sb % 2][ti][po : po + 64, :], cu[0:64, :], rb[:]
                        )

                return [
                    (
                        (lambda b0=b0: sg(b0)),
                        (lambda b0=b0: pg(b0)),
                    )
                    for b0 in range(0, nkb, 4)
                ]

            def make_o_thunks(sb):
                thunks = []
                for qb in range(4):
                    for n2 in range(2):

                        def group(qb=qb, n2=n2, sb=sb):
                            cfc = cfs[sb % 2]
                            pp = psp.tile([128, SQ], f32, name="pp", tag="pp", bufs=2)
                            for fc in range(4):
                                nc.tensor.matmul(
                                    pp[:],
                                    cfc[fc][:, qb * 128 : (qb + 1) * 128],
                                    wo_sb[fc][:, n2 * SQ : (n2 + 1) * SQ],
                                    start=(fc == 0),
                                    stop=(fc == 3),
                                )
                            ob = st.tile([128, SQ], f32, name="ob", tag="ob", bufs=2)
                            nc.vector.tensor_copy(ob[:], pp[:])
                            nc.sync.dma_start(
                                out_d[
                                    sb * SQ + qb * 128 : sb * SQ + (qb + 1) * 128,
                                    n2 * SQ : (n2 + 1) * SQ,
                                ],
                                ob[:],
                            )

                        thunks.append(group)
                return thunks

            # ---- emission schedule ----
            # startup: priority-ordered DMA so the first projection matmuls
            # aren't stuck behind 5MB of resident loads
            load_bias(bk_t, bk_d)
            load_w("k")
            for t in make_proj_thunks(0, only="k"):
                t()
            load_w("v")
            for t in make_proj_thunks(0, only="v"):
                t()
            load_bias(bq_t, bq_d)
            load_w("q")
            for t in make_proj_thunks(0, only="q"):
                t()
            load_misc()
            for sb in range(NSC):
                pairs = []
                for h in range(HL):
                    pairs += make_attn_stream(h, sb)
                # flatten with one-group PV lag: SG_g ; PG_{g-1} ; ...
                batches = [pairs[0][0]]
                for g in range(1, len(pairs)):
                    batches.append(pairs[g][0])
                    batches.append(pairs[g - 1][1])
                batches.append(pairs[-1][1])
                warm = []
                if sb < NSC - 1:
                    warm += make_proj_thunks(sb + 1)
                if sb >= 1:
                    warm += make_o_thunks(sb - 1)
                nb, nw = len(batches), len(warm)
                wi = 0
                for bi, bt in enumerate(batches):
                    bt()
                    while wi < nw and (wi + 1) * nb <= (bi + 1) * nw:
                        warm[wi]()
                        wi += 1
                while wi < nw:
                    warm[wi]()
                    wi += 1
            for t in make_o_thunks(NSC - 1):
                t()

    nc.compile()
    return nc


def kernel(
    q,
    k,
    v,
    mask=None,
    Wq=None,
    bq=None,
    Wk=None,
    bk=None,
    Wv=None,
    bv=None,
    Wo=None,
    bo=None,
    **_unused,
):
    import ml_dtypes

    from concourse.bass_utils import run_bass_kernel_spmd

    if "nc" not in _cache:
        _cache["nc"] = _build()
    nc = _cache["nc"]

    bf16 = ml_dtypes.bfloat16
    q = np.asarray(q, np.float32)
    k = np.asarray(k, np.float32)
    v = np.asarray(v, np.float32)
    Wq = np.asarray(Wq, np.float32)
    Wk = np.asarray(Wk, np.float32)
    Wv = np.asarray(Wv, np.float32)
    Wo = np.asarray(Wo, np.float32)
    bq = np.zeros(D, np.float32) if bq is None else np.asarray(bq, np.float32)
    bk = np.zeros(D, np.float32) if bk is None else np.asarray(bk, np.float32)
    bv = np.zeros(D, np.float32) if bv is None else np.asarray(bv, np.float32)
    bo = np.zeros(D, np.float32) if bo is None else np.asarray(bo, np.float32)

    ident = np.eye(128, dtype=np.float32).astype(bf16)
    kk = np.arange(128)[:, None]
    qq = np.arange(128)[None, :]
    maskb = np.where(kk <= qq, 0.0, NEGB).astype(bf16)

    xT = {}
    for b in range(B):
        xT[("q", b)] = np.ascontiguousarray(q[b].T).astype(bf16)
        xT[("k", b)] = np.ascontiguousarray(k[b].T).astype(bf16)
        xT[("v", b)] = np.ascontiguousarray(v[b].T).astype(bf16)
    wqs, wks, wvs, wos, bqs, bks = {}, {}, {}, {}, {}, {}
    for hg in range(2):
        sl = slice(hg * FH, (hg + 1) * FH)
        wqs[hg] = (np.ascontiguousarray(Wq[sl, :].T) * np.float32(0.125)).astype(bf16)
        wks[hg] = np.ascontiguousarray(Wk[sl, :].T).astype(bf16)
        wvs[hg] = np.ascontiguousarray(Wv[sl, :].T).astype(bf16)
        wos[hg] = np.ascontiguousarray(Wo[:, sl].T).astype(bf16)
        bqs[hg] = np.ascontiguousarray(bq[sl]) * np.float32(0.125)
        bks[hg] = np.ascontiguousarray(bk[sl])

    in_maps = []
    for c in range(NCORES):
        b, hg = c // 2, c % 2
        in_maps.append(
            {
                "xq": xT[("q", b)],
                "xk": xT[("k", b)],
                "xv": xT[("v", b)],
                "wq": wqs[hg],
                "wk": wks[hg],
                "wv": wvs[hg],
                "wo": wos[hg],
                "ident": ident,
                "maskb": maskb,
                "bq": bqs[hg],
                "bk": bks[hg],
            }
        )

    res = run_bass_kernel_spmd(nc, in_maps, list(range(NCORES)))
    out = np.empty((B, S, D), np.float32)
    for b in range(B):
        out[b] = res.results[2 * b]["out"] + res.results[2 * b + 1]["out"]
    const = Wo @ bv + bo  # bv/bo contribution (folds exactly through softmax)
    if np.any(const):
        out += const[None, None, :]
    return out


# revision 16
# speedup vs baseline: 1.2383x; 1.0004x over previous
"""Multi-head causal attention (B=4, S=2048, D=1024, H=16) on 8 trn2 cores.

Sharding: tensor-parallel over heads x data-parallel over batch.
core c -> (batch b = c//2, head-group hg = c%2 of 8 heads). Every core runs
an identical SPMD program on different data; the host sums the two partial
outputs per batch (the "all-reduce after W_o" done at gather time) and folds
the Wo @ bv + bo constant.

v3 design (vs the 480us f32r baseline):
  - All matmul inputs bf16 (tolerance 2e-2; this lands ~5e-3).
  - Score matmuls for a head PAIR run concurrently in the PE array via
    row tiling: head 2t uses array rows 0-63 (tile_position (0,0)), head
    2t+1 rows 64-127 ((64,0)) - auto-derived from the operands' base
    partitions. Halves score matmul wall time; K/Q tiles need no padding.
  - Weights + K^T/V~/Q/ctx all SBUF-resident; weights load once; ctx never
    round-trips through DRAM.
  - Causal trimming: for diagonal key-block j only query columns >= 128*j
    are computed anywhere (scores/mask/exp/PV). The fine triangular mask is
    applied by accumulating identity.T @ maskbias (0/-30) into the score
    PSUM before exp - no vector-engine mask multiply.
  - Softmax denominators come free as PV row 64 via a ones column in each
    head's V block (stationary M=65). The PV accumulator bank is released
    by a single DVE copy (ctx+denominator -> SBUF); the normalize chain
    (reciprocal, gpsimd broadcast, multiply) then runs entirely off-PSUM
    on otherwise-idle engines. Odd heads' normalized ctx is written with a
    cross-quadrant DVE write (HW-verified) to pack ctx^T for the output
    projection.
  - Q/K projection PSUM->SBUF moves ride on the vector engine
    (tensor_scalar_add with the bias), keeping the scalar engine exp-only.
"""

import sys

import numpy as np

_BASS_PATH = "/opt/trn_rl_repo"
if _BASS_PATH not in sys.path:
    sys.path.insert(0, _BASS_PATH)

B, S, D, H, DK = 4, 2048, 1024, 16, 64
NCORES = 8
FH = 512  # features per core (8 heads)
HL = 8  # local heads
NSC = 4  # seq superblocks of 512
SQ = 512
NDM = 8  # d_model chunks of 128
NEGB = -30.0  # causal mask bias (exp(-30+s) ~ 0 for |s|<=8)

_cache = {}


def _build():
    import concourse.bacc as bacc
    import concourse.mybir as mybir
    from concourse.tile import TileContext

    f32, bf16 = mybir.dt.float32, mybir.dt.bfloat16
    AF = mybir.ActivationFunctionType

    nc = bacc.Bacc("TRN2", target_bir_lowering=False, debug=False, num_devices=1)

    xq_d = nc.dram_tensor("xq", [D, S], bf16, kind="ExternalInput").ap()
    xk_d = nc.dram_tensor("xk", [D, S], bf16, kind="ExternalInput").ap()
    xv_d = nc.dram_tensor("xv", [D, S], bf16, kind="ExternalInput").ap()
    wq_d = nc.dram_tensor("wq", [D, FH], bf16, kind="ExternalInput").ap()
    wk_d = nc.dram_tensor("wk", [D, FH], bf16, kind="ExternalInput").ap()
    wv_d = nc.dram_tensor("wv", [D, FH], bf16, kind="ExternalInput").ap()
    wo_d = nc.dram_tensor("wo", [FH, D], bf16, kind="ExternalInput").ap()
    id_d = nc.dram_tensor("ident", [128, 128], bf16, kind="ExternalInput").ap()
    mb_d = nc.dram_tensor("maskb", [128, 128], bf16, kind="ExternalInput").ap()
    bq_d = nc.dram_tensor("bq", [FH], f32, kind="ExternalInput").ap()
    bk_d = nc.dram_tensor("bk", [FH], f32, kind="ExternalInput").ap()
    out_d = nc.dram_tensor("out", [S, D], f32, kind="ExternalOutput").ap()

    with TileContext(nc) as tc:
        with (
            tc.tile_pool(name="res", bufs=1) as res,
            tc.tile_pool(name="st", bufs=1) as st,
            tc.tile_pool(name="psum", bufs=1, space="PSUM") as psp,
        ):
            # ---- resident tiles (DMAs emitted later in priority order) ----
            kt = [res.tile([128, S], bf16, name=f"kt{i}", tag=f"kt{i}") for i in range(4)]
            # V~ per key block: 8 heads x (64 V cols + ones col), stride 65
            vaug = [
                res.tile([128, 520], bf16, name=f"va{k}", tag=f"va{k}")
                for k in range(16)
            ]
            wk_sb = [res.tile([128, FH], bf16, name=f"wk{i}", tag=f"wk{i}") for i in range(NDM)]
            wv_sb = [res.tile([128, FH], bf16, name=f"wv{i}", tag=f"wv{i}") for i in range(NDM)]
            wq_sb = [res.tile([128, FH], bf16, name=f"wq{i}", tag=f"wq{i}") for i in range(NDM)]
            w_res = {"k": wk_sb, "v": wv_sb, "q": wq_sb}
            w_dram = {"k": wk_d, "v": wv_d, "q": wq_d}
            wo_sb = [
                res.tile([128, D], bf16, name=f"wo{fc}", tag=f"wo{fc}")
                for fc in range(4)
            ]
            ident_t = res.tile([128, 128], bf16, name="ident", tag="ident")
            maskb_t = res.tile([128, 128], bf16, name="maskb", tag="maskb")
            bq_t = [res.tile([128, 1], f32, name=f"bq{i}", tag=f"bq{i}") for i in range(4)]
            bk_t = [res.tile([128, 1], f32, name=f"bk{i}", tag=f"bk{i}") for i in range(4)]
            ones_t = res.tile([128, HL], bf16, name="ones", tag="ones")
            nc.vector.memset(ones_t[:], 1.0)

            def load_w(pname):
                for dm in range(NDM):
                    nc.sync.dma_start(
                        w_res[pname][dm][:],
                        w_dram[pname][dm * 128 : (dm + 1) * 128, :],
                    )

            def load_bias(bt, b_d):
                for i in range(4):
                    nc.sync.dma_start(
                        bt[i][:],
                        b_d[i * 128 : (i + 1) * 128].rearrange("(p o) -> p o", o=1),
                    )

            def load_misc():
                nc.sync.dma_start(ident_t[:], id_d[:])
                nc.sync.dma_start(maskb_t[:], mb_d[:])
                for fc in range(4):
                    nc.sync.dma_start(wo_sb[fc][:], wo_d[fc * 128 : (fc + 1) * 128, :])
            # per-head zero-padded Q tiles, double-buffered over superblocks:
            # head h data lives in rows (h%2)*64 .. +64, other 64 rows are 0,
            # so score matmuls contract K=128 (keeps the HAM clock-gate warm)
            qz = [
                [
                    res.tile([128, SQ], bf16, name=f"qz{s}_{h}", tag=f"qz{s}_{h}")
                    for h in range(HL)
                ]
                for s in range(2)
            ]
            for s in range(2):
                for h in range(HL):
                    zr = 64 if (h % 2 == 0) else 0
                    nc.vector.memset(qz[s][h][zr : zr + 64, :], 0.0)
            # SBUF-resident ctx^T (features x queries), double-buffered
            cfs = [
                [
                    res.tile([128, SQ], bf16, name=f"cf{s}_{fc}", tag=f"cf{s}_{fc}")
                    for fc in range(4)
                ]
                for s in range(2)
            ]

            def load_x(sc, x_d, box):
                xr = []
                for dm in range(NDM):
                    xt = st.tile(
                        [128, SQ], bf16, name=f"x{dm}", tag=f"x{dm}", bufs=1
                    )
                    nc.sync.dma_start(
                        xt[:],
                        x_d[dm * 128 : (dm + 1) * 128, sc * SQ : (sc + 1) * SQ],
                    )
                    xr.append(xt)
                box["x"] = xr

            def make_proj_thunks(sc, only=None, box0=None):
                thunks = []
                for pname, x_d in (("k", xk_d), ("v", xv_d), ("q", xq_d)):
                    if only is not None and pname != only:
                        continue
                    box = box0 if box0 is not None else {}

                    def load(pname=pname, x_d=x_d, box=box):
                        if "x" in box:
                            return
                        load_x(sc, x_d, box)

                    for gi in range(4):

                        def group(pname=pname, gi=gi, box=box, sc=sc, load=load):
                            if gi == 0:
                                load()
                            w_sb = w_res[pname]
                            xr = box["x"]
                            if pname in ("q", "k"):
                                pp = psp.tile(
                                    [128, SQ], f32, name="pp", tag="pp", bufs=2
                                )
                                for dm in range(NDM):
                                    nc.tensor.matmul(
                                        pp[:],
                                        w_sb[dm][:, gi * 128 : (gi + 1) * 128],
                                        xr[dm][:],
                                        start=(dm == 0),
                                        stop=(dm == NDM - 1),
                                    )
                                if pname == "k":
                                    nc.vector.tensor_scalar_add(
                                        kt[gi][:, sc * SQ : (sc + 1) * SQ],
                                        pp[:],
                                        bk_t[gi][:],
                                    )
                                else:
                                    s = sc % 2
                                    nc.vector.tensor_scalar_add(
                                        qz[s][2 * gi][0:64, :],
                                        pp[0:64, :],
                                        bq_t[gi][0:64],
                                    )
                                    nc.vector.tensor_scalar_add(
                                        qz[s][2 * gi + 1][64:128, :],
                                        pp[64:128, :],
                                        bq_t[gi][64:128],
                                    )
                            else:  # v
                                kb = sc * 4 + gi
                                pp = psp.tile(
                                    [128, FH], f32, name="pp", tag="pp", bufs=2
                                )
                                for dm in range(NDM):
                                    nc.tensor.matmul(
                                        pp[:],
                                        xr[dm][:, gi * 128 : (gi + 1) * 128],
                                        w_sb[dm][:],
                                        start=(dm == 0),
                                        stop=(dm == NDM - 1),
                                    )
                                va3 = vaug[kb][:, 0 : HL * 65].rearrange(
                                    "p (h e) -> p h e", e=65
                                )
                                pp3 = pp[:].rearrange("p (h e) -> p h e", e=64)
                                nc.vector.tensor_copy(va3[:, :, 0:64], pp3[:])
                                nc.vector.tensor_copy(
                                    va3[:, :, 64:65],
                                    ones_t[:].rearrange("p (h o) -> p h o", o=1),
                                )

                        thunks.append(group)
                return thunks

            def make_attn_stream(h, sb):
                """(score-group, pv-group) thunk pairs for one (head, sb).

                Score groups (QK matmuls + mask + exp) and PV groups are
                emitted with a one-group lag by the scheduler so the PE never
                waits on the scalar engine's exp latency.
                """
                ti = h // 2
                nkb = 4 * (sb + 1)
                # off-diagonal blocks first (full N), then diagonal ascending
                kbs = list(range(4 * sb)) + list(range(4 * sb, 4 * sb + 4))
                state = {}

                def sg(b0):
                    group = []
                    for i in range(b0, b0 + 4):
                        kb = kbs[i]
                        j = kb - 4 * sb  # >=0 on diagonal blocks
                        c0 = 128 * j if j >= 0 else 0
                        sp = psp.tile([128, SQ], f32, name="sp", tag="sp", bufs=4)
                        nc.tensor.matmul(
                            sp[:, c0:SQ],
                            kt[ti][:, kb * 128 : (kb + 1) * 128],
                            qz[sb % 2][h][:, c0:SQ],
                            start=True,
                            stop=(j < 0),
                        )
                        if j >= 0:
                            nc.tensor.matmul(
                                sp[:, c0 : c0 + 128],
                                ident_t[:],
                                maskb_t[:],
                                start=False,
                                stop=True,
                            )
                        es = st.tile([128, SQ], bf16, name="es", tag="es", bufs=8)
                        nc.scalar.activation(es[:, c0:SQ], sp[:, c0:SQ], AF.Exp)
                        group.append((kb, c0, es))
                    state[b0] = group

                def pg(b0):
                    if b0 == 0:
                        state["cp"] = psp.tile(
                            [128, SQ], f32, name="cp", tag="cp", bufs=2
                        )
                        state["emitted"] = 0
                    cp = state["cp"]
                    for kb, c0, es in state.pop(b0):
                        nc.tensor.matmul(
                            cp[0:65, c0:SQ],
                            vaug[kb][:, h * 65 : h * 65 + 65],
                            es[:, c0:SQ],
                            start=(state["emitted"] == 0),
                            stop=(state["emitted"] == nkb - 1),
                        )
                        state["emitted"] += 1
                    if b0 + 4 >= nkb:
                        po = (h % 2) * 64
                        if sb == NSC - 1 and h >= HL - 2:
                            # kernel tail: shorter chain (scalar is idle by
                            # now and nothing else needs the PSUM bank)
                            d1 = st.tile([1, SQ], f32, name="d1", tag="d1", bufs=4)
                            nc.scalar.copy(d1[:], cp[64:65, :])
                            rc1 = st.tile(
                                [1, SQ], f32, name="rc1", tag="rc1", bufs=4
                            )
                            nc.vector.reciprocal_approx_fast(rc1[:], d1[:])
                            rb = st.tile([64, SQ], f32, name="rb", tag="rb", bufs=4)
                            nc.gpsimd.partition_broadcast(rb[:], rc1[:])
                            nc.vector.tensor_mul(
                                cfs[sb % 2][ti][po : po + 64, :], cp[0:64, :], rb[:]
                            )
                            return
                        # single DVE copy releases the PSUM bank; the rest of
                        # the normalize chain runs off SBUF on idle engines
                        cu = st.tile([65, SQ], f32, name="cu", tag="cu", bufs=4)
                        nc.vector.tensor_copy(cu[:], cp[0:65, :])
                        d1 = st.tile([1, SQ], f32, name="d1", tag="d1", bufs=4)
                        nc.vector.tensor_copy(d1[:], cu[64:65, :])
                        rc1 = st.tile([1, SQ], f32, name="rc1", tag="rc1", bufs=4)
                        nc.vector.reciprocal_approx_fast(rc1[:], d1[:])
                        rb = st.tile([64, SQ], f32, name="rb", tag="rb", bufs=4)
                        nc.gpsimd.partition_broadcast(rb[:], rc1[:])
                        nc.vector.tensor_mul(
                            cfs[sb % 2][ti][po : po + 64, :], cu[0:64, :], rb[:]
                        )

                return [
                    (
                        (lambda b0=b0: sg(b0)),
                        (lambda b0=b0: pg(b0)),
                    )
                    for b0 in range(0, nkb, 4)
                ]

            def make_o_thunks(sb):
                thunks = []
                for qb in range(4):
                    for n2 in range(2):

                        def group(qb=qb, n2=n2, sb=sb):
                            cfc = cfs[sb % 2]
                            pp = psp.tile([128, SQ], f32, name="pp", tag="pp", bufs=2)
                            for fc in range(4):
                                nc.tensor.matmul(
                                    pp[:],
                                    cfc[fc][:, qb * 128 : (qb + 1) * 128],
                                    wo_sb[fc][:, n2 * SQ : (n2 + 1) * SQ],
                                    start=(fc == 0),
                                    stop=(fc == 3),
                                )
                            ob = st.tile([128, SQ], f32, name="ob", tag="ob", bufs=2)
                            if sb == NSC - 1:
                                # scalar engine is exp-free by the last
                                # superblock; keep the DVE out of the tail
                                nc.scalar.copy(ob[:], pp[:])
                            else:
                                nc.vector.tensor_copy(ob[:], pp[:])
                            nc.sync.dma_start(
                                out_d[
                                    sb * SQ + qb * 128 : sb * SQ + (qb + 1) * 128,
                                    n2 * SQ : (n2 + 1) * SQ,
                                ],
                                ob[:],
                            )

                        thunks.append(group)
                return thunks

            # ---- emission schedule ----
            # startup: priority-ordered DMA so the first projection matmuls
            # aren't stuck behind 5MB of resident loads; wk/xk descriptors
            # interleave so matmul dm can start as soon as chunk dm lands
            boxk = {}
            xk_tiles = []
            for dm in range(NDM):
                nc.sync.dma_start(
                    wk_sb[dm][:], wk_d[dm * 128 : (dm + 1) * 128, :]
                )
                xt = st.tile([128, SQ], bf16, name=f"x{dm}", tag=f"x{dm}", bufs=1)
                nc.sync.dma_start(xt[:], xk_d[dm * 128 : (dm + 1) * 128, 0:SQ])
                xk_tiles.append(xt)
            boxk["x"] = xk_tiles
            load_bias(bk_t, bk_d)
            for t in make_proj_thunks(0, only="k", box0=boxk):
                t()
            load_w("v")
            for t in make_proj_thunks(0, only="v"):
                t()
            load_bias(bq_t, bq_d)
            load_w("q")
            for t in make_proj_thunks(0, only="q"):
                t()
            load_misc()
            for sb in range(NSC):
                pairs = []
                for h in range(HL):
                    pairs += make_attn_stream(h, sb)
                # flatten with one-group PV lag: SG_g ; PG_{g-1} ; ...
                batches = [pairs[0][0]]
                for g in range(1, len(pairs)):
                    batches.append(pairs[g][0])
                    batches.append(pairs[g - 1][1])
                batches.append(pairs[-1][1])
                warm = []
                if sb < NSC - 1:
                    warm += make_proj_thunks(sb + 1)
                if sb >= 1:
                    warm += make_o_thunks(sb - 1)
                nb, nw = len(batches), len(warm)
                wi = 0
                for bi, bt in enumerate(batches):
                    bt()
                    while wi < nw and (wi + 1) * nb <= (bi + 1) * nw:
                        warm[wi]()
                        wi += 1
                while wi < nw:
                    warm[wi]()
                    wi += 1
            for t in make_o_thunks(NSC - 1):
                t()

    nc.compile()
    return nc


def kernel(
    q,
    k,
    v,
    mask=None,
    Wq=None,
    bq=None,
    Wk=None,
    bk=None,
    Wv=None,
    bv=None,
    Wo=None,
    bo=None,
    **_unused,
):
    import ml_dtypes

    from concourse.bass_utils import run_bass_kernel_spmd

    if "nc" not in _cache:
        _cache["nc"] = _build()
    nc = _cache["nc"]

    bf16 = ml_dtypes.bfloat16
    q = np.asarray(q, np.float32)
    k = np.asarray(k, np.float32)
    v = np.asarray(v, np.float32)
    Wq = np.asarray(Wq, np.float32)
    Wk = np.asarray(Wk, np.float32)
    Wv = np.asarray(Wv, np.float32)
    Wo = np.asarray(Wo, np.float32)
    bq = np.zeros(D, np.float32) if bq is None else np.asarray(bq, np.float32)
    bk = np.zeros(D, np.float32) if bk is None else np.asarray(bk, np.float32)
    bv = np.zeros(D, np.float32) if bv is None else np.asarray(bv, np.float32)
    bo = np.zeros(D, np.float32) if bo is None else np.asarray(bo, np.float32)

    ident = np.eye(128, dtype=np.float32).astype(bf16)
    kk = np.arange(128)[:, None]
    qq = np.arange(128)[None, :]
    maskb = np.where(kk <= qq, 0.0, NEGB).astype(bf16)

    xT = {}
    for b in range(B):
        xT[("q", b)] = np.ascontiguousarray(q[b].T).astype(bf16)
        xT[("k", b)] = np.ascontiguousarray(k[b].T).astype(bf16)
        xT[("v", b)] = np.ascontiguousarray(v[b].T).astype(bf16)
    wqs, wks, wvs, wos, bqs, bks = {}, {}, {}, {}, {}, {}
    for hg in range(2):
        sl = slice(hg * FH, (hg + 1) * FH)
        wqs[hg] = (np.ascontiguousarray(Wq[sl, :].T) * np.float32(0.125)).astype(bf16)
        wks[hg] = np.ascontiguousarray(Wk[sl, :].T).astype(bf16)
        wvs[hg] = np.ascontiguousarray(Wv[sl, :].T).astype(bf16)
        wos[hg] = np.ascontiguousarray(Wo[:, sl].T).astype(bf16)
        bqs[hg] = np.ascontiguousarray(bq[sl]) * np.float32(0.125)
        bks[hg] = np.ascontiguousarray(bk[sl])

    in_maps = []
    for c in range(NCORES):
        b, hg = c // 2, c % 2
        in_maps.append(
            {
                "xq": xT[("q", b)],
                "xk": xT[("k", b)],
                "xv": xT[("v", b)],
                "wq": wqs[hg],
                "wk": wks[hg],
                "wv": wvs[hg],
                "wo": wos[hg],
                "ident": ident,
                "maskb": maskb,
                "bq": bqs[hg],
                "bk": bks[hg],
            }
        )

    res = run_bass_kernel_spmd(nc, in_maps, list(range(NCORES)))
    out = np.empty((B, S, D), np.float32)
    for b in range(B):
        out[b] = res.results[2 * b]["out"] + res.results[2 * b + 1]["out"]
    const = Wo @ bv + bo  # bv/bo contribution (folds exactly through softmax)
    if np.any(const):
        out += const[None, None, :]
    return out
